# revision 2
# baseline (speedup 1.0000x reference)
"""Multi-head attention (MemoryNet) Bass kernel for 8 Trainium2 cores.

Problem (per reference):
  q,k: [b=4, d=1024, m/n=2048], v: [4, 1024, 2048] fp32, N_HEAD=8
  per head (32 total): S = (qh^T kh)/sqrt(128); P = softmax(S, axis=-1)
  out_head = vh @ P^T  -> [128, 2048]; out = [4, 1024, 2048]

Sharding: 32 heads = 8 cores x 4 heads; pure head parallelism.

Per-core kernel (per head):
  - load q,k,v head slices fp32, cast bf16
  - vT via DMA-xbar transposes, with an appended ones-column
  - S^T[n,m] = k^T q on TensorE (lhsT=k-tile, rhs=q), fp32 PSUM
  - exp(scale*S^T) on ScalarE PSUM->SBUF bf16 (no max subtraction: scores
    bounded ~ +-8, softmax shift-invariant)
  - O^T[m, c+1] = expS^T.T @ [vT | 1] accumulated over 16 n-chunks; the
    ones column gives Z[m] (softmax denominator) in the same psum
  - normalize by 1/Z (per-partition scalar), PE-transpose to [c, m], fp32 out
"""

import sys

sys.path.insert(0, "/opt/trn_rl_repo")

import numpy as np

N_CORES = 8
HPC = 4  # heads per core
DH = 128  # head dim (contraction for QK)
M = 2048  # queries
NK = 2048  # keys
CH = 128  # v channels per head
NT = NK // 128  # 16 n-chunks
MT = M // 128  # 16 m-tiles
SCALE = 1.0 / float(np.sqrt(DH))

_CACHE = {}


def _build():
    from contextlib import ExitStack

    from concourse import bacc, mybir, tile
    from concourse.masks import make_identity

    f32 = mybir.dt.float32
    bf16 = mybir.dt.bfloat16

    nc = bacc.Bacc("TRN2", target_bir_lowering=False, debug=False,
                   num_devices=N_CORES)
    q4 = nc.dram_tensor("q4", (HPC, DH, M), f32, kind="ExternalInput").ap()
    k4 = nc.dram_tensor("k4", (HPC, DH, NK), f32, kind="ExternalInput").ap()
    v4 = nc.dram_tensor("v4", (HPC, CH, NK), f32, kind="ExternalInput").ap()
    o4 = nc.dram_tensor("o4", (HPC, CH, M), f32, kind="ExternalOutput").ap()

    with tile.TileContext(nc) as tc, ExitStack() as ctx:
        stage = ctx.enter_context(tc.tile_pool(name="stage", bufs=4))
        bfp = ctx.enter_context(tc.tile_pool(name="bfp", bufs=6))
        vtp = ctx.enter_context(tc.tile_pool(name="vtp", bufs=2))
        ep = ctx.enter_context(tc.tile_pool(name="ep", bufs=18))
        outp = ctx.enter_context(tc.tile_pool(name="outp", bufs=2))
        smallp = ctx.enter_context(tc.tile_pool(name="smallp", bufs=8))
        onep = ctx.enter_context(tc.tile_pool(name="onep", bufs=1))
        pss = ctx.enter_context(tc.tile_pool(name="pss", bufs=2, space="PSUM"))
        pso = ctx.enter_context(tc.tile_pool(name="pso", bufs=2, space="PSUM"))
        pst = ctx.enter_context(tc.tile_pool(name="pst", bufs=2, space="PSUM"))

        ident = onep.tile([128, 128], bf16)
        make_identity(nc, ident)

        for h in range(HPC):
            # ---- load + cast ----
            qf = stage.tile([DH, M], f32, tag="stage")
            nc.sync.dma_start(out=qf, in_=q4[h])
            kf = stage.tile([DH, NK], f32, tag="stage")
            nc.sync.dma_start(out=kf, in_=k4[h])
            vf = stage.tile([CH, NK], f32, tag="stage")
            nc.sync.dma_start(out=vf, in_=v4[h])
            qb = bfp.tile([DH, M], bf16, tag="bf")
            nc.vector.tensor_copy(qb, qf)
            kb = bfp.tile([DH, NK], bf16, tag="bf")
            nc.vector.tensor_copy(kb, kf)
            vb = bfp.tile([CH, NK], bf16, tag="bf")
            nc.vector.tensor_copy(vb, vf)

            # ---- vT with ones column: [128(n-chunk), NT, 132] ----
            vton = vtp.tile([128, NT, 132], bf16, tag="vt")
            nc.gpsimd.memset(vton, 1.0)
            for j in range(NT):
                # xbar transpose needs a contiguous dst; copy into the packed
                # [vT | ones] layout afterwards
                vtj = smallp.tile([128, 128], bf16, tag="vtj")
                nc.sync.dma_start_transpose(
                    out=vtj, in_=vb[:, 128 * j:128 * (j + 1)]
                )
                nc.vector.tensor_copy(vton[:, j, 0:128], vtj)

            # ---- S^T = k^T q, exp -> expst (bf16) ----
            expst = []
            for j in range(NT):
                e = ep.tile([128, M], bf16, tag="e")
                kslice = kb[:, 128 * j:128 * (j + 1)]
                for half in range(2):
                    s = pss.tile([128, 1024], f32, tag="s")
                    for quarter in range(2):
                        mo = 1024 * half + 512 * quarter
                        nc.tensor.matmul(
                            s[:, 512 * quarter:512 * (quarter + 1)],
                            kslice,
                            qb[:, mo:mo + 512],
                            start=True,
                            stop=True,
                        )
                    nc.scalar.activation(
                        e[:, 1024 * half:1024 * (half + 1)],
                        s,
                        mybir.ActivationFunctionType.Exp,
                        scale=SCALE,
                    )
                expst.append(e)

            # ---- AV: O^T[m-tile, 132] accum over n-chunks; col128 = Z ----
            out_sb = outp.tile([CH, M], f32, tag="out")
            packs = [list(range(p, min(p + 3, MT))) for p in range(0, MT, 3)]
            for pack in packs:
                po = pso.tile([128, 3, 132], f32, tag="po")
                for t, i in enumerate(pack):
                    for j in range(NT):
                        nc.tensor.matmul(
                            po[:, t, :],
                            expst[j][:, 128 * i:128 * (i + 1)],
                            vton[:, j, :],
                            start=(j == 0),
                            stop=(j == NT - 1),
                        )
                for t, i in enumerate(pack):
                    rz = smallp.tile([128, 1], f32, tag="rz")
                    nc.vector.reciprocal(rz, po[:, t, 128:129])
                    otn = smallp.tile([128, 128], bf16, tag="otn")
                    nc.vector.tensor_scalar_mul(otn, po[:, t, 0:128], rz)
                    pt = pst.tile([128, 128], bf16, tag="pt")
                    nc.tensor.transpose(pt, otn, ident)
                    nc.vector.tensor_copy(out_sb[:, 128 * i:128 * (i + 1)], pt)

            nc.sync.dma_start(out=o4[h], in_=out_sb)

    nc.compile()
    return nc


def _get_nc():
    if "nc" not in _CACHE:
        _CACHE["nc"] = _build()
    return _CACHE["nc"]


def kernel(q, k, v):
    from concourse.bass_utils import run_bass_kernel_spmd

    nc = _get_nc()
    b, d, m = q.shape
    qh = np.ascontiguousarray(q.reshape(32, DH, M))
    kh = np.ascontiguousarray(k.reshape(32, DH, NK))
    vh = np.ascontiguousarray(v.reshape(32, CH, NK))
    in_maps = [
        {
            "q4": qh[HPC * c:HPC * (c + 1)],
            "k4": kh[HPC * c:HPC * (c + 1)],
            "v4": vh[HPC * c:HPC * (c + 1)],
        }
        for c in range(N_CORES)
    ]
    res = run_bass_kernel_spmd(nc, in_maps, core_ids=list(range(N_CORES)))
    out = np.concatenate([res.results[c]["o4"] for c in range(N_CORES)], axis=0)
    return out.reshape(b, d, m).astype(np.float32)
